# revision 2
# baseline (speedup 1.0000x reference)
"""Trainium2 Bass kernel for nn_AlpacaImitation (log-predictive posterior head).

Math (per batch row b, per u):
    phi_b = MLP(x_b)                               (PHI=128 vector, tanh MLP)
    sigfactor[b,u] = 1 + phi_b^T L[b,u] phi_b
    mu[b,u]        = Q[b,u,0,:] . (L[b,u] @ phi_b)
    sig[b,u,:,:]   = diag(exp(logSigEps)) * sigfactor[b,u]

Distribution: pure data parallel over the batch dim B=1024 across 8 cores
(128 rows/core); MLP weights + logSigEps replicated.

Per-core device algorithm (all heavy traffic is the 67 MB L shard):
  - MLP runs on TensorE with transposed activations; phivT (PHI x b).
  - L[b] streams through the PE as the bf16 moving operand (SWDGE DMA does
    the fp32->bf16 cast inline):  P[m, (u,q)] = RQ[:, b]^T @ L[b]  where
    RQ columns are [phi_b, Q_b0..Q_b7].  Four b's share one PSUM tile with
    9-row blocks at partition offsets {0,32,64,96} (col-tiled matmuls).
  - Second contraction on VectorE: multiply by phi (broadcast via
    selection-matrix matmuls, all precomputed in the prologue) and reduce
    over q per u -> res8[128, g, u].
  - sig diag = (1 + res8 rows 32a) * exp(logSigEps) on device; mu is the
    [32a+1+u, g, u] diagonal of res8, gathered on the host (pure indexing).
"""

import numpy as np

from concourse import bacc, mybir, tile
from concourse.bass_utils import run_bass_kernel_spmd

B, X_DIM, U_DIM, PHI, HID = 1024, 32, 8, 128, 128
NCORES = 8
BC = B // NCORES          # 128 batch rows per core
GROUPS = BC // 4          # 32 groups of 4 b's (one PSUM tile each)
DMA_B = 8                 # b's per L DMA chunk
NCHUNK = BC // DMA_B      # 16 L DMA chunks per core

_cache = {}


def _build():
    nc = bacc.Bacc("TRN2", target_bir_lowering=False)
    f32 = mybir.dt.float32
    bf16 = mybir.dt.bfloat16

    L_ext = nc.declare_dram_parameter("L", [BC, U_DIM, PHI, PHI], f32, isOutput=False)
    x_ext = nc.declare_dram_parameter("x", [BC, X_DIM], f32, isOutput=False)
    Q_ext = nc.declare_dram_parameter("Q", [BC, U_DIM, PHI], f32, isOutput=False)
    W1_ext = nc.declare_dram_parameter("W1", [X_DIM, HID], f32, isOutput=False)
    W2_ext = nc.declare_dram_parameter("W2", [HID, HID], f32, isOutput=False)
    W3_ext = nc.declare_dram_parameter("W3", [HID, HID], f32, isOutput=False)
    W4_ext = nc.declare_dram_parameter("W4", [HID, PHI], f32, isOutput=False)
    b1_ext = nc.declare_dram_parameter("b1", [HID, 1], f32, isOutput=False)
    b2_ext = nc.declare_dram_parameter("b2", [HID, 1], f32, isOutput=False)
    b3_ext = nc.declare_dram_parameter("b3", [HID, 1], f32, isOutput=False)
    b4_ext = nc.declare_dram_parameter("b4", [PHI, 1], f32, isOutput=False)
    eps_ext = nc.declare_dram_parameter("logSigEps", [1, U_DIM], f32, isOutput=False)
    id_ext = nc.declare_dram_parameter("ident", [128, 128], f32, isOutput=False)
    sel_ext = nc.declare_dram_parameter("sel", [128, GROUPS, 128], f32, isOutput=False)
    ones_ext = nc.declare_dram_parameter("ones1", [1, 128], f32, isOutput=False)

    res8_ext = nc.declare_dram_parameter("out_res8", [128, GROUPS, U_DIM], f32, isOutput=True)
    sigd_ext = nc.declare_dram_parameter("out_sigd", [128, GROUPS, U_DIM], f32, isOutput=True)

    with tile.TileContext(nc) as tc:
        with (
            tc.tile_pool(name="const", bufs=1) as cpool,
            tc.tile_pool(name="work", bufs=1) as wpool,
            tc.tile_pool(name="lt", bufs=4) as lpool,
            tc.tile_pool(name="tmp", bufs=3) as tpool,
            tc.tile_pool(name="pps", bufs=2, space="PSUM") as pps,
            tc.tile_pool(name="ppp", bufs=3, space="PSUM") as ppp,
        ):
            # ---------- constant / small input loads ----------
            xsb = cpool.tile([BC, X_DIM], f32, tag="xsb")
            nc.sync.dma_start(xsb[:], x_ext[:])
            w1 = cpool.tile([X_DIM, HID], f32, tag="w1")
            nc.sync.dma_start(w1[:], W1_ext[:])
            w2 = cpool.tile([HID, HID], f32, tag="w2")
            nc.sync.dma_start(w2[:], W2_ext[:])
            w3 = cpool.tile([HID, HID], f32, tag="w3")
            nc.sync.dma_start(w3[:], W3_ext[:])
            w4 = cpool.tile([HID, PHI], f32, tag="w4")
            nc.sync.dma_start(w4[:], W4_ext[:])
            b1t = cpool.tile([HID, 1], f32, tag="b1t")
            nc.sync.dma_start(b1t[:], b1_ext[:])
            b2t = cpool.tile([HID, 1], f32, tag="b2t")
            nc.sync.dma_start(b2t[:], b2_ext[:])
            b3t = cpool.tile([HID, 1], f32, tag="b3t")
            nc.sync.dma_start(b3t[:], b3_ext[:])
            b4t = cpool.tile([PHI, 1], f32, tag="b4t")
            nc.sync.dma_start(b4t[:], b4_ext[:])
            ident = cpool.tile([128, 128], f32, tag="ident")
            nc.sync.dma_start(ident[:], id_ext[:])
            tsel = cpool.tile([128, GROUPS, 128], f32, tag="tsel")
            nc.sync.dma_start(tsel[:], sel_ext[:])
            tones = cpool.tile([1, 128], f32, tag="tones")
            nc.sync.dma_start(tones[:], ones_ext[:])
            teps = cpool.tile([1, U_DIM], f32, tag="teps")
            nc.sync.dma_start(teps[:], eps_ext[:])
            qsb = cpool.tile([BC, U_DIM, PHI], f32, tag="qsb")
            nc.sync.dma_start(qsb[:], Q_ext[:])

            # ---------- MLP: phivT = W4^T tanh(W3^T tanh(W2^T tanh(W1^T x^T + b1) + b2) + b3) + b4
            pxt = pps.tile([X_DIM, BC], f32, tag="pps")
            nc.tensor.transpose(pxt[:], xsb[:], ident[:])
            xT = wpool.tile([X_DIM, BC], f32, tag="xT")
            nc.scalar.copy(xT[:], pxt[:])

            ph1 = pps.tile([HID, BC], f32, tag="pps")
            nc.tensor.matmul(ph1[:], w1[:], xT[:], start=True, stop=True)
            h1 = wpool.tile([HID, BC], f32, tag="h1")
            nc.scalar.activation(h1[:], ph1[:], mybir.ActivationFunctionType.Tanh, bias=b1t[:])

            ph2 = pps.tile([HID, BC], f32, tag="pps")
            nc.tensor.matmul(ph2[:], w2[:], h1[:], start=True, stop=True)
            h2 = wpool.tile([HID, BC], f32, tag="h2")
            nc.scalar.activation(h2[:], ph2[:], mybir.ActivationFunctionType.Tanh, bias=b2t[:])

            ph3 = pps.tile([HID, BC], f32, tag="pps")
            nc.tensor.matmul(ph3[:], w3[:], h2[:], start=True, stop=True)
            h3 = wpool.tile([HID, BC], f32, tag="h3")
            nc.scalar.activation(h3[:], ph3[:], mybir.ActivationFunctionType.Tanh, bias=b3t[:])

            ph4 = pps.tile([PHI, BC], f32, tag="pps")
            nc.tensor.matmul(ph4[:], w4[:], h3[:], start=True, stop=True)
            phivT = wpool.tile([PHI, BC], f32, tag="phivT")
            nc.vector.tensor_scalar_add(phivT[:], ph4[:], b4t[:])

            # phiv (b on partitions)
            pphiv = pps.tile([BC, PHI], f32, tag="pps")
            nc.tensor.transpose(pphiv[:], phivT[:], ident[:])
            phiv = wpool.tile([BC, PHI], f32, tag="phiv")
            nc.scalar.copy(phiv[:], pphiv[:])

            # ---------- RQ (bf16): columns [phi_b, Q_b0..Q_b7] per b ----------
            RQ = wpool.tile([PHI, BC, 9], bf16, tag="RQ")
            nc.vector.tensor_copy(RQ[:, :, 0], phivT[:])
            for u in range(U_DIM):
                pqt = pps.tile([PHI, BC], f32, tag="pps")
                nc.tensor.transpose(pqt[:], qsb[:, u, :], ident[:])
                nc.vector.tensor_copy(RQ[:, :, 1 + u], pqt[:])

            # ---------- eps broadcast: epsb[m, u] = exp(logSigEps[u]) ----------
            epse = wpool.tile([1, U_DIM], f32, tag="epse")
            nc.scalar.activation(epse[:], teps[:], mybir.ActivationFunctionType.Exp)
            pep = pps.tile([128, U_DIM], f32, tag="pps")
            nc.tensor.matmul(pep[:], tones[:], epse[:], start=True, stop=True)
            epsb = wpool.tile([128, U_DIM], f32, tag="epsb")
            nc.scalar.copy(epsb[:], pep[:])

            # ---------- all mult tiles in the prologue ----------
            # mult_all[32a+j, g, q] = phi_{4g+a}[q]
            mult_all = wpool.tile([128, GROUPS, PHI], f32, tag="mult_all")
            for g in range(GROUPS):
                pm = pps.tile([128, PHI], f32, tag="pps")
                nc.tensor.matmul(pm[:], tsel[:, g, :], phiv[:], start=True, stop=True)
                nc.scalar.copy(mult_all[:, g, :], pm[:])

            # ---------- main loop over L ----------
            res8 = wpool.tile([128, GROUPS, U_DIM], f32, tag="res8")

            for d in range(NCHUNK):
                Lt = lpool.tile([128, DMA_B, U_DIM, PHI], bf16, tag="Lt")
                src = L_ext[d * DMA_B : (d + 1) * DMA_B].transpose([2, 0, 1, 3])
                nc.gpsimd.dma_start(Lt[:], src)  # SWDGE casts fp32->bf16 inline

                for gg in range(DMA_B // 4):
                    g = d * (DMA_B // 4) + gg
                    pp = ppp.tile([128, U_DIM * PHI], f32, tag="pp")
                    for a in range(4):
                        bcol = 4 * g + a
                        for h in range(2):
                            nc.tensor.matmul(
                                pp[32 * a : 32 * a + 9, 512 * h : 512 * (h + 1)],
                                RQ[:, bcol, :],
                                Lt[:, 4 * gg + a, 4 * h : 4 * (h + 1), :],
                                start=True,
                                stop=True,
                                tile_position=(0, 32 * a),
                            )

                    tmp = tpool.tile([128, U_DIM, PHI], f32, tag="tmp")
                    nc.vector.tensor_tensor(
                        tmp[:],
                        pp[:].rearrange("p (u q) -> p u q", q=PHI),
                        mult_all[:, g, :].unsqueeze(1).broadcast_to([128, U_DIM, PHI]),
                        op=mybir.AluOpType.mult,
                    )
                    nc.vector.tensor_reduce(
                        res8[:, g, :], tmp[:], axis=mybir.AxisListType.X,
                        op=mybir.AluOpType.add,
                    )

            # ---------- epilogue: sig diag values ----------
            t1 = wpool.tile([128, GROUPS, U_DIM], f32, tag="t1")
            nc.vector.tensor_scalar_add(t1[:], res8[:], 1.0)
            sigd = wpool.tile([128, GROUPS, U_DIM], f32, tag="sigd")
            nc.vector.tensor_tensor(
                sigd[:], t1[:],
                epsb[:].unsqueeze(1).broadcast_to([128, GROUPS, U_DIM]),
                op=mybir.AluOpType.mult,
            )

            # ---------- outputs: bulk, host does the (pure-index) gather ----------
            nc.sync.dma_start(res8_ext[:], res8[:])
            nc.sync.dma_start(sigd_ext[:], sigd[:])

    nc.compile()
    return nc


def _consts():
    sel = np.zeros((128, GROUPS, 128), dtype=np.float32)
    for g in range(GROUPS):
        for a in range(4):
            sel[4 * g + a, g, 32 * a : 32 * a + 32] = 1.0
    return {
        "ident": np.eye(128, dtype=np.float32),
        "sel": sel,
        "ones1": np.ones((1, 128), dtype=np.float32),
    }


# host-side gather indices: mu[4g+a, u] = res8[32a+1+u, g, u];
# sigd row: sig_diag[4g+a, u] = sigd[32a, g, u]
_g_idx = np.arange(GROUPS)[:, None, None]          # g
_a_idx = np.arange(4)[None, :, None]               # a
_u_idx = np.arange(U_DIM)[None, None, :]           # u
_MU_PART = (32 * _a_idx + 1 + _u_idx)              # (1,4,8) broadcast w/ g
_SIG_PART = (32 * _a_idx) + 0 * _u_idx


def _run(inputs, trace=False):
    if "nc" not in _cache:
        _cache["nc"] = _build()
        _cache["consts"] = _consts()
    nc = _cache["nc"]
    consts = _cache["consts"]

    x = np.ascontiguousarray(np.asarray(inputs["x"], dtype=np.float32))
    Q = np.ascontiguousarray(
        np.asarray(inputs["Q"], dtype=np.float32).reshape(B, U_DIM, PHI)
    )
    L = np.ascontiguousarray(np.asarray(inputs["L"], dtype=np.float32))
    rep = {
        "W1": np.asarray(inputs["W1"], np.float32),
        "W2": np.asarray(inputs["W2"], np.float32),
        "W3": np.asarray(inputs["W3"], np.float32),
        "W4": np.asarray(inputs["W4"], np.float32),
        "b1": np.asarray(inputs["b1"], np.float32).reshape(HID, 1),
        "b2": np.asarray(inputs["b2"], np.float32).reshape(HID, 1),
        "b3": np.asarray(inputs["b3"], np.float32).reshape(HID, 1),
        "b4": np.asarray(inputs["b4"], np.float32).reshape(PHI, 1),
        "logSigEps": np.asarray(inputs["logSigEps"], np.float32).reshape(1, U_DIM),
        **consts,
    }

    in_maps = []
    for c in range(NCORES):
        sl = slice(c * BC, (c + 1) * BC)
        in_maps.append({"L": L[sl], "x": x[sl], "Q": Q[sl], **rep})

    res = run_bass_kernel_spmd(nc, in_maps, core_ids=list(range(NCORES)), trace=trace)

    mu = np.empty((B, U_DIM), dtype=np.float32)
    sig_diag = np.empty((B, U_DIM), dtype=np.float32)
    for c in range(NCORES):
        r8 = res.results[c]["out_res8"]       # (128, GROUPS, U)
        sd = res.results[c]["out_sigd"]       # (128, GROUPS, U)
        mu_c = r8[_MU_PART, _g_idx, _u_idx]   # (GROUPS, 4, U)
        sd_c = sd[_SIG_PART, _g_idx, _u_idx]
        mu[c * BC : (c + 1) * BC] = mu_c.reshape(BC, U_DIM)
        sig_diag[c * BC : (c + 1) * BC] = sd_c.reshape(BC, U_DIM)

    mu = mu.reshape(B, U_DIM, 1)
    sig = np.zeros((B, U_DIM, U_DIM), dtype=np.float32)
    idx = np.arange(U_DIM)
    sig[:, idx, idx] = sig_diag
    return (mu, sig), res


def kernel(**inputs):
    (mu, sig), _ = _run(inputs, trace=False)
    return mu, sig


# revision 3
# speedup vs baseline: 1.6268x; 1.6268x over previous
"""Trainium2 Bass kernel for nn_AlpacaImitation (log-predictive posterior head).

Math (per batch row b, per u):
    phi_b = MLP(x_b)                               (PHI=128 vector, tanh MLP)
    sigfactor[b,u] = 1 + phi_b^T L[b,u] phi_b
    mu[b,u]        = Q[b,u,0,:] . (L[b,u] @ phi_b)
    sig[b,u,:,:]   = diag(exp(logSigEps)) * sigfactor[b,u]

Distribution: pure data parallel over the batch dim B=1024 across 8 cores
(128 rows/core); MLP weights + logSigEps replicated.

Per-core device algorithm (all heavy traffic is the 67 MB L shard):
  - MLP runs on TensorE with transposed activations; phivT (PHI x b).
  - L[b] streams through the PE as the bf16 moving operand (SWDGE DMA does
    the fp32->bf16 cast inline):  P[m, (u,q)] = RQ[:, b]^T @ L[b]  where
    RQ columns are [phi_b, Q_b0..Q_b7].  Four b's share one PSUM tile with
    9-row blocks at partition offsets {0,32,64,96} (col-tiled matmuls).
  - Second contraction on VectorE: multiply by phi (broadcast via
    selection-matrix matmuls, all precomputed in the prologue) and reduce
    over q per u -> res8[128, g, u].
  - sig diag = (1 + res8 rows 32a) * exp(logSigEps) on device; mu is the
    [32a+1+u, g, u] diagonal of res8, gathered on the host (pure indexing).
"""

import numpy as np

from concourse import bacc, mybir, tile
from concourse.bass_utils import run_bass_kernel_spmd

B, X_DIM, U_DIM, PHI, HID = 1024, 32, 8, 128, 128
NCORES = 8
BC = B // NCORES          # 128 batch rows per core
GROUPS = BC // 4          # 32 groups of 4 b's (one PSUM tile each)
DMA_B = 4                 # b's per L DMA chunk
NCHUNK = BC // DMA_B      # 16 L DMA chunks per core

_cache = {}


def _build():
    nc = bacc.Bacc("TRN2", target_bir_lowering=False)
    f32 = mybir.dt.float32
    bf16 = mybir.dt.bfloat16

    L_ext = nc.declare_dram_parameter("L", [BC, U_DIM, PHI, PHI], f32, isOutput=False)
    x_ext = nc.declare_dram_parameter("x", [BC, X_DIM], f32, isOutput=False)
    Q_ext = nc.declare_dram_parameter("Q", [BC, U_DIM, PHI], f32, isOutput=False)
    W1_ext = nc.declare_dram_parameter("W1", [X_DIM, HID], f32, isOutput=False)
    W2_ext = nc.declare_dram_parameter("W2", [HID, HID], f32, isOutput=False)
    W3_ext = nc.declare_dram_parameter("W3", [HID, HID], f32, isOutput=False)
    W4_ext = nc.declare_dram_parameter("W4", [HID, PHI], f32, isOutput=False)
    b1_ext = nc.declare_dram_parameter("b1", [HID, 1], f32, isOutput=False)
    b2_ext = nc.declare_dram_parameter("b2", [HID, 1], f32, isOutput=False)
    b3_ext = nc.declare_dram_parameter("b3", [HID, 1], f32, isOutput=False)
    b4_ext = nc.declare_dram_parameter("b4", [PHI, 1], f32, isOutput=False)
    eps_ext = nc.declare_dram_parameter("logSigEps", [1, U_DIM], f32, isOutput=False)
    id_ext = nc.declare_dram_parameter("ident", [128, 128], f32, isOutput=False)
    sel_ext = nc.declare_dram_parameter("sel", [128, GROUPS, 128], f32, isOutput=False)
    ones_ext = nc.declare_dram_parameter("ones1", [1, 128], f32, isOutput=False)

    res8_ext = nc.declare_dram_parameter("out_res8", [128, GROUPS, U_DIM], f32, isOutput=True)
    sigd_ext = nc.declare_dram_parameter("out_sigd", [128, GROUPS, U_DIM], f32, isOutput=True)

    with tile.TileContext(nc) as tc:
        with (
            tc.tile_pool(name="const", bufs=1) as cpool,
            tc.tile_pool(name="work", bufs=1) as wpool,
            tc.tile_pool(name="lt", bufs=8) as lpool,
            tc.tile_pool(name="tmp", bufs=3) as tpool,
            tc.tile_pool(name="pps", bufs=2, space="PSUM") as pps,
            tc.tile_pool(name="ppp", bufs=3, space="PSUM") as ppp,
        ):
            # ---------- constant / small input loads ----------
            xsb = cpool.tile([BC, X_DIM], f32, tag="xsb")
            nc.sync.dma_start(xsb[:], x_ext[:])
            w1 = cpool.tile([X_DIM, HID], f32, tag="w1")
            nc.sync.dma_start(w1[:], W1_ext[:])
            w2 = cpool.tile([HID, HID], f32, tag="w2")
            nc.sync.dma_start(w2[:], W2_ext[:])
            w3 = cpool.tile([HID, HID], f32, tag="w3")
            nc.sync.dma_start(w3[:], W3_ext[:])
            w4 = cpool.tile([HID, PHI], f32, tag="w4")
            nc.sync.dma_start(w4[:], W4_ext[:])
            b1t = cpool.tile([HID, 1], f32, tag="b1t")
            nc.sync.dma_start(b1t[:], b1_ext[:])
            b2t = cpool.tile([HID, 1], f32, tag="b2t")
            nc.sync.dma_start(b2t[:], b2_ext[:])
            b3t = cpool.tile([HID, 1], f32, tag="b3t")
            nc.sync.dma_start(b3t[:], b3_ext[:])
            b4t = cpool.tile([PHI, 1], f32, tag="b4t")
            nc.sync.dma_start(b4t[:], b4_ext[:])
            ident = cpool.tile([128, 128], f32, tag="ident")
            nc.sync.dma_start(ident[:], id_ext[:])
            tsel = cpool.tile([128, GROUPS, 128], f32, tag="tsel")
            nc.sync.dma_start(tsel[:], sel_ext[:])
            tones = cpool.tile([1, 128], f32, tag="tones")
            nc.sync.dma_start(tones[:], ones_ext[:])
            teps = cpool.tile([1, U_DIM], f32, tag="teps")
            nc.sync.dma_start(teps[:], eps_ext[:])
            qsb = cpool.tile([BC, U_DIM, PHI], f32, tag="qsb")
            nc.sync.dma_start(qsb[:], Q_ext[:])

            # ---------- MLP: phivT = W4^T tanh(W3^T tanh(W2^T tanh(W1^T x^T + b1) + b2) + b3) + b4
            pxt = pps.tile([X_DIM, BC], f32, tag="pps")
            nc.tensor.transpose(pxt[:], xsb[:], ident[:])
            xT = wpool.tile([X_DIM, BC], f32, tag="xT")
            nc.scalar.copy(xT[:], pxt[:])

            ph1 = pps.tile([HID, BC], f32, tag="pps")
            nc.tensor.matmul(ph1[:], w1[:], xT[:], start=True, stop=True)
            h1 = wpool.tile([HID, BC], f32, tag="h1")
            nc.scalar.activation(h1[:], ph1[:], mybir.ActivationFunctionType.Tanh, bias=b1t[:])

            ph2 = pps.tile([HID, BC], f32, tag="pps")
            nc.tensor.matmul(ph2[:], w2[:], h1[:], start=True, stop=True)
            h2 = wpool.tile([HID, BC], f32, tag="h2")
            nc.scalar.activation(h2[:], ph2[:], mybir.ActivationFunctionType.Tanh, bias=b2t[:])

            ph3 = pps.tile([HID, BC], f32, tag="pps")
            nc.tensor.matmul(ph3[:], w3[:], h2[:], start=True, stop=True)
            h3 = wpool.tile([HID, BC], f32, tag="h3")
            nc.scalar.activation(h3[:], ph3[:], mybir.ActivationFunctionType.Tanh, bias=b3t[:])

            ph4 = pps.tile([PHI, BC], f32, tag="pps")
            nc.tensor.matmul(ph4[:], w4[:], h3[:], start=True, stop=True)
            phivT = wpool.tile([PHI, BC], f32, tag="phivT")
            nc.vector.tensor_scalar_add(phivT[:], ph4[:], b4t[:])

            # phiv (b on partitions)
            pphiv = pps.tile([BC, PHI], f32, tag="pps")
            nc.tensor.transpose(pphiv[:], phivT[:], ident[:])
            phiv = wpool.tile([BC, PHI], f32, tag="phiv")
            nc.scalar.copy(phiv[:], pphiv[:])

            # ---------- RQ (bf16): columns [phi_b, Q_b0..Q_b7] per b ----------
            RQ = wpool.tile([PHI, BC, 9], bf16, tag="RQ")
            nc.vector.tensor_copy(RQ[:, :, 0], phivT[:])
            for u in range(U_DIM):
                pqt = pps.tile([PHI, BC], f32, tag="pps")
                nc.tensor.transpose(pqt[:], qsb[:, u, :], ident[:])
                nc.vector.tensor_copy(RQ[:, :, 1 + u], pqt[:])

            # ---------- eps broadcast: epsb[m, u] = exp(logSigEps[u]) ----------
            epse = wpool.tile([1, U_DIM], f32, tag="epse")
            nc.scalar.activation(epse[:], teps[:], mybir.ActivationFunctionType.Exp)
            pep = pps.tile([128, U_DIM], f32, tag="pps")
            nc.tensor.matmul(pep[:], tones[:], epse[:], start=True, stop=True)
            epsb = wpool.tile([128, U_DIM], f32, tag="epsb")
            nc.scalar.copy(epsb[:], pep[:])

            # ---------- all mult tiles in the prologue ----------
            # mult_all[32a+j, g, q] = phi_{4g+a}[q]
            mult_all = wpool.tile([128, GROUPS, PHI], f32, tag="mult_all")
            for g in range(GROUPS):
                pm = pps.tile([128, PHI], f32, tag="pps")
                nc.tensor.matmul(pm[:], tsel[:, g, :], phiv[:], start=True, stop=True)
                nc.scalar.copy(mult_all[:, g, :], pm[:])

            # ---------- main loop over L ----------
            res8 = wpool.tile([128, GROUPS, U_DIM], f32, tag="res8")

            for d in range(NCHUNK):
                Lt = lpool.tile([128, DMA_B, U_DIM, PHI], bf16, tag="Lt")
                src = L_ext[d * DMA_B : (d + 1) * DMA_B].transpose([2, 0, 1, 3])
                nc.gpsimd.dma_start(Lt[:], src)  # SWDGE casts fp32->bf16 inline

                for gg in range(DMA_B // 4):
                    g = d * (DMA_B // 4) + gg
                    pp = ppp.tile([128, U_DIM * PHI], f32, tag="pp")
                    for a in range(4):
                        bcol = 4 * g + a
                        for h in range(2):
                            nc.tensor.matmul(
                                pp[32 * a : 32 * a + 9, 512 * h : 512 * (h + 1)],
                                RQ[:, bcol, :],
                                Lt[:, 4 * gg + a, 4 * h : 4 * (h + 1), :],
                                start=True,
                                stop=True,
                                tile_position=(0, 32 * a),
                            )

                    tmp = tpool.tile([128, U_DIM, PHI], f32, tag="tmp")
                    nc.vector.tensor_tensor(
                        tmp[:],
                        pp[:].rearrange("p (u q) -> p u q", q=PHI),
                        mult_all[:, g, :].unsqueeze(1).broadcast_to([128, U_DIM, PHI]),
                        op=mybir.AluOpType.mult,
                    )
                    nc.vector.tensor_reduce(
                        res8[:, g, :], tmp[:], axis=mybir.AxisListType.X,
                        op=mybir.AluOpType.add,
                    )

            # ---------- epilogue: sig diag values ----------
            t1 = wpool.tile([128, GROUPS, U_DIM], f32, tag="t1")
            nc.vector.tensor_scalar_add(t1[:], res8[:], 1.0)
            sigd = wpool.tile([128, GROUPS, U_DIM], f32, tag="sigd")
            nc.vector.tensor_tensor(
                sigd[:], t1[:],
                epsb[:].unsqueeze(1).broadcast_to([128, GROUPS, U_DIM]),
                op=mybir.AluOpType.mult,
            )

            # ---------- outputs: bulk, host does the (pure-index) gather ----------
            nc.sync.dma_start(res8_ext[:], res8[:])
            nc.sync.dma_start(sigd_ext[:], sigd[:])

    nc.compile()
    return nc


def _consts():
    sel = np.zeros((128, GROUPS, 128), dtype=np.float32)
    for g in range(GROUPS):
        for a in range(4):
            sel[4 * g + a, g, 32 * a : 32 * a + 32] = 1.0
    return {
        "ident": np.eye(128, dtype=np.float32),
        "sel": sel,
        "ones1": np.ones((1, 128), dtype=np.float32),
    }


# host-side gather indices: mu[4g+a, u] = res8[32a+1+u, g, u];
# sigd row: sig_diag[4g+a, u] = sigd[32a, g, u]
_g_idx = np.arange(GROUPS)[:, None, None]          # g
_a_idx = np.arange(4)[None, :, None]               # a
_u_idx = np.arange(U_DIM)[None, None, :]           # u
_MU_PART = (32 * _a_idx + 1 + _u_idx)              # (1,4,8) broadcast w/ g
_SIG_PART = (32 * _a_idx) + 0 * _u_idx


def _run(inputs, trace=False):
    if "nc" not in _cache:
        _cache["nc"] = _build()
        _cache["consts"] = _consts()
    nc = _cache["nc"]
    consts = _cache["consts"]

    x = np.ascontiguousarray(np.asarray(inputs["x"], dtype=np.float32))
    Q = np.ascontiguousarray(
        np.asarray(inputs["Q"], dtype=np.float32).reshape(B, U_DIM, PHI)
    )
    L = np.ascontiguousarray(np.asarray(inputs["L"], dtype=np.float32))
    rep = {
        "W1": np.asarray(inputs["W1"], np.float32),
        "W2": np.asarray(inputs["W2"], np.float32),
        "W3": np.asarray(inputs["W3"], np.float32),
        "W4": np.asarray(inputs["W4"], np.float32),
        "b1": np.asarray(inputs["b1"], np.float32).reshape(HID, 1),
        "b2": np.asarray(inputs["b2"], np.float32).reshape(HID, 1),
        "b3": np.asarray(inputs["b3"], np.float32).reshape(HID, 1),
        "b4": np.asarray(inputs["b4"], np.float32).reshape(PHI, 1),
        "logSigEps": np.asarray(inputs["logSigEps"], np.float32).reshape(1, U_DIM),
        **consts,
    }

    in_maps = []
    for c in range(NCORES):
        sl = slice(c * BC, (c + 1) * BC)
        in_maps.append({"L": L[sl], "x": x[sl], "Q": Q[sl], **rep})

    res = run_bass_kernel_spmd(nc, in_maps, core_ids=list(range(NCORES)), trace=trace)

    mu = np.empty((B, U_DIM), dtype=np.float32)
    sig_diag = np.empty((B, U_DIM), dtype=np.float32)
    for c in range(NCORES):
        r8 = res.results[c]["out_res8"]       # (128, GROUPS, U)
        sd = res.results[c]["out_sigd"]       # (128, GROUPS, U)
        mu_c = r8[_MU_PART, _g_idx, _u_idx]   # (GROUPS, 4, U)
        sd_c = sd[_SIG_PART, _g_idx, _u_idx]
        mu[c * BC : (c + 1) * BC] = mu_c.reshape(BC, U_DIM)
        sig_diag[c * BC : (c + 1) * BC] = sd_c.reshape(BC, U_DIM)

    mu = mu.reshape(B, U_DIM, 1)
    sig = np.zeros((B, U_DIM, U_DIM), dtype=np.float32)
    idx = np.arange(U_DIM)
    sig[:, idx, idx] = sig_diag
    return (mu, sig), res


def kernel(**inputs):
    (mu, sig), _ = _run(inputs, trace=False)
    return mu, sig
